# revision 30
# baseline (speedup 1.0000x reference)
"""Multi-head attention (B=4, S=2048, D=1024, H=16, Hd=64) on 8 TRN2 NeuronCores.

Sharding: tensor-parallel over heads - 2 heads per core (128 channels).
Each core computes its heads' Q/K/V projections, attention, and the partial
output projection (its 128 rows of Wo); the host sums the 8 partials + bo.

Schedule: a single deadline-driven emission loop over 128 global pair-steps
(16 blocks x 8 kt-pairs).  Each pair-step emits two score-pair matmuls + one
1024-wide exp per kt, the AV matmuls for the previous pair, a paced slice of
projection work (units with forced just-in-time deadlines and an even paced
spread over the preceding batch), and a fragment of the previous block's
finalize (denominator broadcast, reciprocal, normalize, output projection).
This keeps the ACT engine (softmax exp, the throughput floor at ~1.04us per
kt) and the PE continuously fed instead of alternating between ACT-paced
attention segments and PE-only projection/finalize bursts.

PSUM budget (8 banks): score tiles [128,1024]x2 (4), AV accumulators
[65,512]x2 (2, own tag so finalize fragments can't deadlock the rotation),
shared scratch [128,512]x2 (2) for projection runs / V transposes /
denominator broadcast / output projection.
"""
import sys

sys.path.insert(0, "/opt/trn_rl_repo")

import numpy as np
import ml_dtypes

import concourse.bass as bass
import concourse.mybir as mybir
import concourse.tile as tile
from concourse import bacc, bass_utils
from concourse.masks import make_identity

B, S, D = 4, 2048, 1024
BS = B * S            # 8192 rows
NCORES = 8
CPC = 128             # channels per core (2 heads x 64)
HD = 64               # head dim
P = 128
QT_TILE = 512         # q-tile width
NQT = BS // QT_TILE   # 16
NKT = S // P          # 16 k-tiles per batch
NQA = S // QT_TILE    # 4 q-tiles per batch
XQ = S                # 2048 rows per x chunk (one batch)
KCH = D // P          # 8 contraction chunks for the projections

F32 = mybir.dt.float32
CD = mybir.dt.bfloat16          # compute dtype on device
CD_NP = ml_dtypes.bfloat16

LAST_RESULTS = None
_NC_CACHE = {}


def build_nc():
    if "nc" in _NC_CACHE:
        return _NC_CACHE["nc"]
    nc = bacc.Bacc(trn_type="TRN2", num_devices=NCORES)

    # x staged host-side in sbuf-partition-major layouts so every DMA
    # descriptor is >=8KB contiguous per partition (the naive (o p) q -> p o q
    # rearrange shatters into 1KB descriptors and loads at ~100 GB/s)
    xh0 = nc.dram_tensor("xh0", [NQA, P, KCH * QT_TILE], CD, kind="ExternalInput").ap()
    xhr = nc.dram_tensor("xhr", [B - 1, P, KCH * XQ], CD, kind="ExternalInput").ap()
    wq = nc.dram_tensor("wq", [D, CPC], CD, kind="ExternalInput").ap()
    wk = nc.dram_tensor("wk", [D, CPC], CD, kind="ExternalInput").ap()
    wv = nc.dram_tensor("wv", [D, CPC], CD, kind="ExternalInput").ap()
    wo = nc.dram_tensor("wo", [CPC, D], CD, kind="ExternalInput").ap()
    bq = nc.dram_tensor("bq", [CPC, 1], F32, kind="ExternalInput").ap()
    bk = nc.dram_tensor("bk", [CPC, 1], F32, kind="ExternalInput").ap()
    bv = nc.dram_tensor("bv", [CPC, 1], F32, kind="ExternalInput").ap()
    y = nc.dram_tensor("y", [BS, D], F32, kind="ExternalOutput").ap()

    scale = float(1.0 / np.sqrt(np.float32(HD)))

    with tile.TileContext(nc) as tc:
        with (
            tc.tile_pool(name="pers", bufs=1) as pers,
            tc.tile_pool(name="xin", bufs=2) as xin,
            tc.tile_pool(name="vtp", bufs=2) as vtp,
            tc.tile_pool(name="pt", bufs=6) as pt,
            tc.tile_pool(name="otu", bufs=4) as otu_pool,
            tc.tile_pool(name="rsb", bufs=2) as rsb_pool,
            tc.tile_pool(name="onp", bufs=2) as on_pool,
            tc.tile_pool(name="yp", bufs=3) as yp,
            tc.tile_pool(name="psS", bufs=2, space="PSUM") as psS,
            tc.tile_pool(name="psOT", bufs=2, space="PSUM") as psOT,
            tc.tile_pool(name="ps2", bufs=2, space="PSUM") as ps2,
        ):
            # ---- persistent tensors ----
            qt_sb = pers.tile([P, BS], CD, tag="QT")
            kt_sb = pers.tile([P, BS], CD, tag="KT")
            v_sb = pers.tile([P, BS // P, 2 * HD + 2], CD, tag="V")
            wq_sb = pers.tile([P, KCH, CPC], CD, tag="wq")
            wk_sb = pers.tile([P, KCH, CPC], CD, tag="wk")
            wv_sb = pers.tile([P, KCH, CPC], CD, tag="wv")
            wo_sb = pers.tile([P, D], CD, tag="wo")
            bq_sb = pers.tile([CPC, 1], F32, tag="bq")
            bk_sb = pers.tile([CPC, 1], F32, tag="bk")
            bv_sb = pers.tile([CPC, 1], F32, tag="bv")
            onesf_sb = pers.tile([P, HD], CD, tag="onesf")
            ident_sb = pers.tile([P, P], CD, tag="ident")

            # only what the first K/Q projections need loads before the
            # first x q-tile; wv/wo/bv stream in later as scheduled units
            nc.sync.dma_start(wk_sb[:], wk.rearrange("(o p) c -> p o c", p=P))
            nc.sync.dma_start(wq_sb[:], wq.rearrange("(o p) c -> p o c", p=P))
            nc.sync.dma_start(bq_sb[:], bq[:, :])
            nc.sync.dma_start(bk_sb[:], bk[:, :])
            nc.vector.memset(onesf_sb[:], 1.0)
            make_identity(nc, ident_sb[:])
            # ones-columns of V_aug (denominator trick), set once for all tiles
            nc.vector.memset(v_sb[:, :, HD : HD + 1], 1.0)
            nc.vector.memset(v_sb[:, :, 2 * HD + 1 : 2 * HD + 2], 1.0)

            def pe_warm(n):
                # dummy ident matmuls keep the PE p-state ramped while it
                # would otherwise idle waiting on a serial DVE chain
                for w in range((n + 31) // 32):
                    wps = psS.tile([P, P], F32, tag="s", name="wps")
                    for _ in range(min(32, n - 32 * w)):
                        nc.tensor.matmul(
                            wps[:], ident_sb[:], ident_sb[:], start=True, stop=True
                        )

            def u_wv():
                nc.sync.dma_start(wv_sb[:], wv.rearrange("(o p) c -> p o c", p=P))
                nc.sync.dma_start(bv_sb[:], bv[:, :])

            def u_wo():
                nc.sync.dma_start(wo_sb[:], wo[:, :])

            # ---------- projection units ----------
            chunk_xt = {}

            def u_dma_chunk(b):
                def run():
                    xt = xin.tile([P, KCH, XQ], CD, tag="xt", name="xt")
                    chunk_xt[b] = xt
                    nc.sync.dma_start(
                        xt[:],
                        xhr[b - 1].rearrange("p (o q) -> p o q", o=KCH),
                    )
                return run

            def u_dma_lq0(b, lq):
                # chunk 0 loads per q-tile, split in two o-halves so the
                # first projection's o0..3 matmuls start before the full
                # tile lands (subtile deps track the o-ranges)
                def run():
                    if lq == 0:
                        xt = xin.tile([P, KCH, XQ], CD, tag="xt", name="xt")
                        chunk_xt[b] = xt
                    xt = chunk_xt[b]
                    l0 = lq * QT_TILE
                    src = xh0[lq].rearrange("p (o q) -> p o q", o=KCH)
                    h = KCH // 2
                    nc.sync.dma_start(
                        xt[:, 0:h, l0 : l0 + QT_TILE], src[:, 0:h, :]
                    )
                    nc.sync.dma_start(
                        xt[:, h:KCH, l0 : l0 + QT_TILE], src[:, h:KCH, :]
                    )
                return run

            def proj_run(b, lq, w_sb, b_sb, dst):
                # one full 9-matmul accumulation run: consecutive same-bank
                # matmuls stream at N/2.4
                xt = chunk_xt[b]
                l0 = lq * QT_TILE
                q0 = b * XQ + l0
                pj = psS.tile([P, QT_TILE], F32, tag="s", name="pj")
                for o in range(KCH):
                    nc.tensor.matmul(
                        pj[:], w_sb[:, o, :], xt[:, o, l0 : l0 + QT_TILE],
                        start=(o == 0), stop=(o == KCH - 1),
                    )
                nc.vector.tensor_scalar_add(
                    dst[:, q0 : q0 + QT_TILE], pj[:], b_sb[:, 0:1]
                )

            def u_K(b, lq):
                return lambda: proj_run(b, lq, wk_sb, bk_sb, kt_sb)

            def u_Q(b, lq):
                return lambda: proj_run(b, lq, wq_sb, bq_sb, qt_sb)

            vt_tiles = {}

            def u_V1(b, lq):
                # VT chunk: projection + bias into a transposed staging tile
                def run():
                    xt = chunk_xt[b]
                    l0 = lq * QT_TILE
                    pj = psS.tile([P, QT_TILE], F32, tag="s", name="pjv")
                    for o in range(KCH):
                        nc.tensor.matmul(
                            pj[:], wv_sb[:, o, :], xt[:, o, l0 : l0 + QT_TILE],
                            start=(o == 0), stop=(o == KCH - 1),
                        )
                    vt_sb = vtp.tile([P, QT_TILE], CD, tag="vt", name="vt")
                    vt_tiles[(b, lq)] = vt_sb
                    nc.vector.tensor_scalar_add(vt_sb[:], pj[:], bv_sb[:, 0:1])
                return run

            def u_V2(b, lq, half):
                # PE-transpose two 128-row groups into natural [seq, ch]
                # layout (SBUF->SBUF DMA-XBAR transpose NaNs on HW)
                def run():
                    vt_sb = vt_tiles[(b, lq)]
                    for rt in range(2 * half, 2 * half + 2):
                        tp = psS.tile([P, P], CD, tag="s", name="tp")
                        nc.tensor.transpose(
                            tp[:], vt_sb[:, rt * P : (rt + 1) * P], ident_sb[:]
                        )
                        grt = (b * XQ + lq * QT_TILE) // P + rt
                        nc.vector.tensor_copy(
                            out=v_sb[:, grt, 0:HD], in_=tp[:, 0:HD]
                        )
                        nc.vector.tensor_copy(
                            out=v_sb[:, grt, HD + 1 : 2 * HD + 1],
                            in_=tp[:, HD:CPC],
                        )
                return run

            # Build the unit list with forced (just-in-time) deadlines in
            # global pair-steps (8 per block, 32 per batch), plus an even
            # paced spread over a window straddling the previous batch.
            units = []
            dues = []

            def add(unit, due):
                units.append(unit)
                dues.append(due)

            # dues are in HALF-steps (2 pull sites per pair-step) so filler
            # units spread one-at-a-time instead of bursting 2x8 matmuls
            # into the PE queue ahead of the next scores
            for b in range(B):
                G2 = 64 * b
                sec = []  # (unit, rel_due_halfsteps, pe_cost_weight)
                for lq in range(NQA):
                    if b == 0:
                        sec.append((u_dma_lq0(b, lq), 4 * lq, 0))
                    elif lq == 0:
                        sec.append((u_dma_chunk(b), 0, 0))
                    sec.append((u_K(b, lq), 4 * lq, 3))
                    if b == 0 and lq == 0:
                        # head: Q before the V units so the first exp isn't
                        # gated on the V transpose chain or the wv/wo loads
                        sec.append((u_Q(b, lq), 0, 3))
                        sec.append((u_wv, 2, 0))
                    sec.append((u_V1(b, lq), 4 * lq + 2, 3))
                    sec.append((u_V2(b, lq, 0), 4 * lq + 2, 1))
                    sec.append((u_V2(b, lq, 1), min(4 * lq + 4, 15), 1))
                    if b == 0 and lq == 0:
                        sec.append((u_wo, 8, 0))
                    else:
                        sec.append((u_Q(b, lq), 16 * lq, 3))
                ctot = sum(c for _, _, c in sec)
                csum = 0
                for unit, rel_due, cost in sec:
                    forced = G2 + rel_due
                    csum += cost
                    if b == 0:
                        due = forced
                    else:
                        # paced window [G2-40, G2+24): straddles the batch
                        # boundary so late batches keep PE filler work
                        paced = G2 - 40 + (64 * csum) // (ctot + 1)
                        due = min(forced, max(paced, 0))
                    add(unit, due)

            # suffix-min so the pull loop (strictly in list order) never
            # stalls behind a later-due unit
            for j in range(len(dues) - 2, -1, -1):
                dues[j] = min(dues[j], dues[j + 1])

            next_u = [0]

            def pull_due(g):
                while next_u[0] < len(units) and dues[next_u[0]] <= g:
                    units[next_u[0]]()
                    next_u[0] += 1

            # ---------- attention emission ----------
            def emit_st_exp(b, qa, kt):
                q0 = b * S + qa * QT_TILE
                k0 = b * S + kt * P
                stp = ps2.tile([P, 2 * QT_TILE], F32, tag="stp", name="stp")
                for h in range(2):
                    hp = h * HD
                    nc.tensor.matmul(
                        stp[:, h * QT_TILE : (h + 1) * QT_TILE],
                        kt_sb[hp : hp + HD, k0 : k0 + P],
                        qt_sb[hp : hp + HD, q0 : q0 + QT_TILE],
                        start=True, stop=True,
                    )
                p_t = pt.tile([P, 2 * QT_TILE], CD, tag="p", name="p")
                nc.scalar.activation(
                    p_t[:], stp[:], mybir.ActivationFunctionType.Exp, scale=scale
                )
                return p_t

            def emit_av_pair(ot, b, kts, ptd):
                for h in range(2):
                    vcol = h * (HD + 1)
                    for kt in kts:
                        nc.tensor.matmul(
                            ot[h][0 : HD + 1, :],
                            v_sb[:, b * NKT + kt, vcol : vcol + HD + 1],
                            ptd[kt][:, h * QT_TILE : (h + 1) * QT_TILE],
                            start=(kt == 0), stop=(kt == NKT - 1),
                        )

            # ---------- finalize fragments (for block state fs) ----------
            def frag_bcast_recip(fs):
                # denominators live in row 64 of the evacuated otu tiles;
                # broadcast them across 64 partitions via K=1 matmuls, then
                # reciprocal straight out of PSUM.  The recips are emitted
                # immediately so the rps slot has no unread window (slot
                # reuse deps are traced in emission order).
                rps = psS.tile([P, QT_TILE], F32, tag="s", name="rps")
                for h in range(2):
                    nc.tensor.matmul(
                        rps[h * HD : (h + 1) * HD, :],
                        onesf_sb[HD : HD + 1, :],
                        fs["otu"][h][HD : HD + 1, :],
                        start=True, stop=True,
                    )
                rsum_sb = rsb_pool.tile(
                    [HD, 2 * QT_TILE], F32, tag="rsum", name="rsum"
                )
                nc.vector.tensor_copy(out=rsum_sb[:, 0:QT_TILE], in_=rps[0:HD, :])
                nc.vector.tensor_copy(out=rsum_sb[:, QT_TILE:], in_=rps[HD:CPC, :])
                r_sb = rsb_pool.tile([HD, 2 * QT_TILE], F32, tag="r", name="r")
                nc.vector.reciprocal_approx_fast(out=r_sb[:], in_=rsum_sb[:])
                fs["r"] = r_sb

            def frag_mul(fs):
                on = on_pool.tile([P, QT_TILE], CD, tag="on", name="on")
                nc.vector.tensor_mul(
                    out=on[0:HD, :], in0=fs["otu"][0][0:HD, :],
                    in1=fs["r"][:, 0:QT_TILE],
                )
                nc.vector.tensor_mul(
                    out=on[HD:CPC, :], in0=fs["otu"][1][0:HD, :],
                    in1=fs["r"][:, QT_TILE:],
                )
                fs["on"] = on

            def frag_outproj(fs, j, evac_act=False):
                q0 = fs["b"] * S + fs["qa"] * QT_TILE
                on = fs["on"]
                ysb = yp.tile([P, D], F32, tag="y", name="ysb")
                for e in range(D // QT_TILE):
                    yps = psS.tile([P, QT_TILE], F32, tag="s", name="yps")
                    nc.tensor.matmul(
                        yps[:],
                        on[:, j * P : (j + 1) * P],
                        wo_sb[:, e * QT_TILE : (e + 1) * QT_TILE],
                        start=True, stop=True,
                    )
                    dst = ysb[:, e * QT_TILE : (e + 1) * QT_TILE]
                    if evac_act and e == 1:
                        # tail only: ACT idles there; evacuate the two psum
                        # halves on ACT and DVE in parallel
                        nc.scalar.activation(
                            dst, yps[:], mybir.ActivationFunctionType.Copy
                        )
                    else:
                        nc.vector.tensor_copy(out=dst, in_=yps[:])
                nc.sync.dma_start(y[q0 + j * P : q0 + (j + 1) * P, :], ysb[:])

            FRAGS = {
                2: [frag_bcast_recip],
                3: [frag_mul],
                4: [lambda fs: frag_outproj(fs, 0)],
                5: [lambda fs: frag_outproj(fs, 1)],
                6: [lambda fs: frag_outproj(fs, 2)],
                7: [lambda fs: frag_outproj(fs, 3)],
            }

            # ---------- main loop ----------
            blocks = [(b, qa) for b in range(B) for qa in range(NQA)]
            fin_state = None
            for bi, (b, qa) in enumerate(blocks):
                pull_due(16 * bi)
                ot = [
                    psOT.tile([P, QT_TILE], F32, tag="ot", name=f"ot{h}")
                    for h in range(2)
                ]
                pts = {}
                for p in range(8):
                    g = 8 * bi + p
                    pull_due(2 * g)
                    k0, k1 = 2 * p, 2 * p + 1
                    # AV lags 2 pair-steps so its p_t inputs are always old
                    # and the matmuls never head-of-line block the PE queue
                    if p >= 2:
                        a0, a1 = k0 - 4, k1 - 4
                        emit_av_pair(
                            ot, b, (a0, a1), {a0: pts.pop(a0), a1: pts.pop(a1)}
                        )
                    pts[k0] = emit_st_exp(b, qa, k0)
                    pts[k1] = emit_st_exp(b, qa, k1)
                    pull_due(2 * g + 1)
                    if fin_state is not None and p in FRAGS:
                        for frag in FRAGS[p]:
                            frag(fin_state)
                for kt in (NKT - 4, NKT - 2):
                    emit_av_pair(
                        ot, b, (kt, kt + 1),
                        {kt: pts.pop(kt), kt + 1: pts.pop(kt + 1)},
                    )
                # evacuate OT psum to SBUF immediately: frees the psum banks
                # and decouples the normalization chain from the accumulators
                otu = [
                    otu_pool.tile(
                        [HD + 1, QT_TILE], CD, tag="otu", name=f"otu{h}"
                    )
                    for h in range(2)
                ]
                for h in range(2):
                    nc.vector.tensor_copy(out=otu[h][:], in_=ot[h][0 : HD + 1, :])
                fin_state = {"b": b, "qa": qa, "otu": otu}
            # tail: finalize the last block.  The warm dummies execute while
            # the rsum/recip/mul DVE chain runs so the output projections
            # start at full clock; their psum evacuation splits across
            # DVE and ACT per e-half.
            frag_bcast_recip(fin_state)
            pe_warm(72)
            frag_mul(fin_state)
            for j in range(NQA):
                frag_outproj(fin_state, j, evac_act=True)

    nc.compile()
    _NC_CACHE["nc"] = nc
    return nc


def make_in_maps(inputs):
    x = np.asarray(inputs["x"], np.float32)
    Wq = np.asarray(inputs["Wq"], np.float32)
    Wk = np.asarray(inputs["Wk"], np.float32)
    Wv = np.asarray(inputs["Wv"], np.float32)
    Wo = np.asarray(inputs["Wo"], np.float32)
    bq = np.asarray(inputs["bq"], np.float32)
    bk = np.asarray(inputs["bk"], np.float32)
    bv = np.asarray(inputs["bv"], np.float32)

    xT = x.reshape(BS, D).T.astype(CD_NP)  # [D, BS]
    # chunk 0 staged q-tile-major, chunks 1-3 chunk-major; both with the
    # sbuf [p, o, q] element order contiguous per partition
    c0 = xT[:, :XQ].reshape(KCH, P, NQA, QT_TILE)
    xh0 = np.ascontiguousarray(c0.transpose(2, 1, 0, 3)).reshape(
        NQA, P, KCH * QT_TILE
    )
    cr = xT[:, XQ:].reshape(KCH, P, B - 1, XQ)
    xhr = np.ascontiguousarray(cr.transpose(2, 1, 0, 3)).reshape(
        B - 1, P, KCH * XQ
    )
    in_maps = []
    for c in range(NCORES):
        sl = slice(c * CPC, (c + 1) * CPC)
        in_maps.append(
            {
                "xh0": xh0,
                "xhr": xhr,
                "wq": np.ascontiguousarray(Wq[:, sl]).astype(CD_NP),
                "wk": np.ascontiguousarray(Wk[:, sl]).astype(CD_NP),
                "wv": np.ascontiguousarray(Wv[:, sl]).astype(CD_NP),
                "wo": np.ascontiguousarray(Wo[sl, :]).astype(CD_NP),
                "bq": np.ascontiguousarray(bq[sl].reshape(CPC, 1)),
                "bk": np.ascontiguousarray(bk[sl].reshape(CPC, 1)),
                "bv": np.ascontiguousarray(bv[sl].reshape(CPC, 1)),
            }
        )
    return in_maps


def kernel(**inputs):
    global LAST_RESULTS
    bo = np.asarray(inputs["bo"], np.float32)
    nc = build_nc()
    in_maps = make_in_maps(inputs)
    res = bass_utils.run_bass_kernel_spmd(nc, in_maps, core_ids=list(range(NCORES)))
    LAST_RESULTS = res
    acc = np.zeros((BS, D), np.float64)
    for r in res.results:
        acc += r["y"].astype(np.float64)
    out = (acc + bo.astype(np.float64)).astype(np.float32)
    return out.reshape(B, S, D)
